# revision 53
# baseline (speedup 1.0000x reference)
"""Trainium2 Bass kernel: sparse 7x7x7 stride-1 max-pool over a 64^3 voxel grid
(MinkowskiEngine semantics) + per-point MLP (1x1 conv -> ReLU -> 1x1 conv ->
sigmoid) * feats.

Strategy (8 NeuronCores, SPMD, no collectives):
  - Shard the dense grid along z: core k owns z in [8k, 8k+8), and processes a
    14-plane z-slab (3-voxel halo each side; halo replicated on host -> no
    cross-core exchange).
  - The HOST builds the dense slab in the exact on-chip layout the kernel
    wants: per x-plane [128 part = ch%128, 2 ch-halves, 14 z, 64 y] with
    empty voxels = -1e30. Pure data marshalling (scatter + transpose + pad);
    all max/matmul arithmetic stays on device.
  - Device streams over the 64 x-planes: contiguous plane load; window-7 max
    on DVE along z via a 5-op pairs-and-singles scheme (A=z-pair maxes,
    A3=triples of pairs, then even/odd outputs from A3 + one raw plane,
    saving 19% FD vs the plain m2/m4/m7 chain), writing into a z8 tile whose
    y-borders are filled by tiny SBUF->SBUF DMAs on the sync queue; window-7
    along y on the padded z8 (m2/m4/m7 chain); window-7 along x via the
    streaming pair scheme A[p]=max(oy2p,oy2p+1), B=A|A', C=B|A'' and
    win(2q)=max(oy[2q-3],C_q), win(2q+1)=max(C_q,oy[2q+4]) -- 2.4 ops/plane
    instead of 3; fused MLP on PE with the second matmul transposed (W2^T as
    weights) so sigmoid lands in [ch, vox] layout, written into pair tiles;
    one dense pair multiply sg * plane-centers on DVE + one pair store.
  - Host gathers the occupied voxels from the dense product planes (reverse
    marshalling) and casts to fp32.

Measured on the 8-core axon TRN2 fleet: HW exec ~397 us (baseline scatter/
transpose design: 870-895 us; first host-marshalled rewrite: 459 us), rel
err 6.4e-3 vs the fp32 reference. The kernel is DVE-bound: ~405 us of
tensor_max work at ~100% vector-engine occupancy (Scalar ~33%, PE ~37%, DMA
queues <50%); pair-batched z/y passes, 2-pair DMA prefetch and a delayed
pair multiply keep everything else off the critical path. Ops compute only
liveness-required rows/cols (zB 5 of 6 rows, m2y/m4y trimmed) in full-size
tiles so SBUF addresses stay fixed. Notes: (1) DVE op throughput is
sensitive to SBUF operand address phase -- pool sizes/order below are an
empirical local optimum; seemingly neutral layout changes (y-pad 70 vs 72,
+32-elem tile stagger, 3-deep prefetch) each cost 80-90 us. (2) GPSIMD
elementwise offload is counterproductive: ~7 us/op and SBUF port contention
slows concurrent DVE ops by ~30%. (3) 5D access patterns with size-1 dims
crash the device; use a double-width innermost dim instead. (4) An odd
(2-byte) operand offset costs ~27% on strided multi-run ops but a full 2x
(hard 1x mode) on flat single-run ops -- keep odd-offset operands in
run-structured APs.
"""

from contextlib import ExitStack
from dataclasses import dataclass

import numpy as np

C = 256
R = 128
SENT = -1.0e30


@dataclass(frozen=True)
class Cfg:
    D: int = 64           # grid extent per axis
    ZS: int = 8           # owned z-planes per core
    NPTS: int = 100000    # total points
    ncores: int = 8
    dt: str = "bfloat16"  # dtype of planes / pooling / matmul inputs
    PB: int = 2           # planes per build batch (z/y pass batching)

    @property
    def ZH(self):
        return self.ZS + 6

    @property
    def YP(self):
        return self.D + 8  # y padded to 72 (4 sentinel cols each side)

    @property
    def NX(self):
        return self.D

    @property
    def PLF(self):
        return 2 * self.ZH * self.D  # free elems per partition per plane

    @property
    def VOX2(self):
        return 2 * self.ZS * self.D  # compact pooled free elems (2h * 8z * 64y)


FULL = Cfg()


def _np_dt(name):
    import ml_dtypes

    return {"bfloat16": ml_dtypes.bfloat16, "float16": np.float16}[name]


def build_nc(cfg: Cfg):
    """Build the (SPMD, per-core-identical) Bass program."""
    import concourse.bacc as bacc
    import concourse.tile as tile
    from concourse import mybir

    AF = mybir.ActivationFunctionType
    f32 = mybir.dt.float32
    dt = getattr(mybir.dt, cfg.dt)

    D, ZS, ZH, YP, NX, PB = cfg.D, cfg.ZS, cfg.ZH, cfg.YP, cfg.NX, cfg.PB
    PLF = cfg.PLF
    VOX2 = cfg.VOX2
    VOXH = ZS * D  # voxels per x-plane owned (512)
    G = 2 * PB     # merged (plane, half) dim in batched views

    nc = bacc.Bacc("TRN2", target_bir_lowering=False, debug=False,
                   enable_asserts=False, num_devices=cfg.ncores)

    planes = nc.dram_tensor("planes", [NX * 128, PLF], dt, kind="ExternalInput").ap()
    w1 = nc.dram_tensor("w1", [C, R], dt, kind="ExternalInput").ap()
    w2 = nc.dram_tensor("w2", [R, C], dt, kind="ExternalInput").ap()
    outp = nc.dram_tensor("outp", [NX * 128, VOX2], dt, kind="ExternalOutput").ap()

    with tile.TileContext(nc) as tc, ExitStack() as ctx:
        const = ctx.enter_context(tc.tile_pool(name="const", bufs=1))
        pp = ctx.enter_context(tc.tile_pool(name="pp", bufs=6))
        ztp = ctx.enter_context(tc.tile_pool(name="ztp", bufs=2))
        ytp = ctx.enter_context(tc.tile_pool(name="ytp", bufs=2))
        oyp = ctx.enter_context(tc.tile_pool(name="oyp", bufs=4))
        axp = ctx.enter_context(tc.tile_pool(name="axp", bufs=3))
        bxp = ctx.enter_context(tc.tile_pool(name="bxp", bufs=3))
        cxp = ctx.enter_context(tc.tile_pool(name="cxp", bufs=3))
        pxp = ctx.enter_context(tc.tile_pool(name="pxp", bufs=4))
        hpp = ctx.enter_context(tc.tile_pool(name="hpp", bufs=2, space="PSUM"))
        y2p = ctx.enter_context(tc.tile_pool(name="y2p", bufs=2, space="PSUM"))
        hsp = ctx.enter_context(tc.tile_pool(name="hsp", bufs=3))
        sgp = ctx.enter_context(tc.tile_pool(name="sgp", bufs=3))
        prp = ctx.enter_context(tc.tile_pool(name="prp", bufs=2))

        # ---- constants (allocate first so SBUF layout is stable; the
        # weight DMAs are issued after the first plane loads below)
        w1sb = const.tile([128, 2 * R], dt)
        w2sb = const.tile([128, C], dt)
        neg = const.tile([128, VOX2], dt)
        w1v = w1sb[:].rearrange("p (h r) -> p h r", h=2)

        # rings hold APs (slices of pool tiles); missing entries mean -inf,
        # handled by aliasing (max(x, -inf) = x)
        oy_t, oyP_t = {}, {}
        A_t, B_t, C_t = {}, {}, {}

        Pm_t = {}  # pair plane views for the final multiply
        Pp_t = {}  # prefetched pair tiles
        sg_t = {}  # pair sigmoid tiles awaiting the delayed multiply

        def max2(pool, a, b):
            """tensor_max with -inf aliasing: either operand may be None."""
            if a is None:
                return b
            if b is None:
                return a
            t = pool.tile([128, VOX2], dt)
            nc.vector.tensor_max(t[:], a, b)
            return t[:]

        def emit_mlp(k, pxa):
            # ---- MLP on plane k: h = relu(W1^T @ px) on PE+ACT
            pxv = pxa.rearrange("p (h v) -> p h v", h=2)
            hp = hpp.tile([128, VOXH], f32, space="PSUM")
            for h in (0, 1):
                nc.tensor.matmul(
                    hp[:], w1v[:, h, :], pxv[:, h, :],
                    start=(h == 0), stop=(h == 1),
                )
            hs = hsp.tile([128, VOXH], dt)
            nc.scalar.activation(hs[:], hp[:], AF.Relu)
            # ---- y2^T = W2^T @ h: output lands [ch-part, vox]
            y2 = y2p.tile([128, 2 * VOXH], f32, space="PSUM")
            for h in (0, 1):
                nc.tensor.matmul(
                    y2[:, h * VOXH:(h + 1) * VOXH],
                    w2sb[:, h * 128:(h + 1) * 128], hs[:],
                    start=True, stop=True,
                )
            if k % 2 == 0:
                sg_t[k // 2] = sgp.tile([128, 2 * VOX2], dt, name="sgP")
            sgP = sg_t[k // 2]
            nc.scalar.activation(
                sgP[:, (k % 2) * VOX2:(k % 2 + 1) * VOX2], y2[:], AF.Sigmoid
            )

        def load_pair(pi):
            P = pp.tile([128, PB * PLF], dt)
            for b in range(PB):
                x = pi * PB + b
                nc.sync.dma_start(
                    P[:, b * PLF:(b + 1) * PLF],
                    planes[x * 128:(x + 1) * 128, :],
                )
            Pm_t[pi] = P[:]
            Pp_t[pi] = P

        for pi0 in range(2):
            load_pair(pi0)
        # weight loads + sentinel memset issue behind the first plane loads
        nc.sync.dma_start(
            w1sb[:].rearrange("p (h r) -> p h r", h=2),
            w1.rearrange("(h p) r -> p h r", p=128),
        )
        nc.sync.dma_start(w2sb[:], w2)
        nc.gpsimd.memset(neg[:], SENT)

        for i in range(NX + 5):
            if i < NX and i % PB == 0:
                pi = i // PB
                if pi + 2 < NX // PB:
                    load_pair(pi + 2)
                P = Pp_t.pop(pi)
                # batched view: (pl h) merges into one uniform dim of 2*PB
                Pv = P[:].rearrange("p (g z y) -> p g z y", g=G, z=ZH)

                # ---- z-pass (window 7 over ZH=14 -> ZS=8) via z-pairs:
                #   A[m] = max(P[2m], P[2m+1]);  B[m] = max(A[m], A[m+1])
                #   A3[m] = max(B[m], A[m+2])   (= P[2m..2m+5])
                #   z8[2m]   = max(A3[m], P[2m+6])
                #   z8[2m+1] = max(P[2m+1], A3[m+1])
                P4 = P[:].rearrange("p (g m y2) -> p g m y2", g=G, m=ZH // 2)
                zA = ztp.tile([128, G * 7 * D], dt)
                zAv = zA[:].rearrange("p (g m y) -> p g m y", g=G, m=7)
                nc.vector.tensor_max(
                    zAv, P4[:, :, :, 0:D], P4[:, :, :, D:2 * D]
                )
                zB = ztp.tile([128, G * 6 * D], dt)
                zBv = zB[:].rearrange("p (g m y) -> p g m y", g=G, m=6)
                nc.vector.tensor_max(
                    zBv[:, :, 0:5, :], zAv[:, :, 0:5, :], zAv[:, :, 1:6, :]
                )
                zA3 = ztp.tile([128, G * 5 * D], dt)
                zA3v = zA3[:].rearrange("p (g m y) -> p g m y", g=G, m=5)
                nc.vector.tensor_max(
                    zA3v, zBv[:, :, 0:5, :], zAv[:, :, 2:7, :]
                )
                z8 = ztp.tile([128, G * ZS * YP], dt)
                z8v = z8[:].rearrange("p (g z y) -> p g z y", g=G, z=ZS)
                z8r = z8[:].rearrange("p (g m y2) -> p g m y2", g=G, m=ZS // 2)
                negb = neg[:, 0:G * ZS * 4].rearrange(
                    "p (g z y) -> p g z y", g=G, z=ZS)
                nc.scalar.dma_start(z8v[:, :, :, 0:4], negb)
                nc.scalar.dma_start(z8v[:, :, :, 4 + D:YP], negb)
                nc.vector.tensor_max(
                    z8r[:, :, :, 4:4 + D],
                    zA3v[:, :, 0:4, :], P4[:, :, 3:7, 0:D],
                )
                nc.vector.tensor_max(
                    z8r[:, :, :, YP + 4:YP + 4 + D],
                    P4[:, :, 0:4, D:2 * D], zA3v[:, :, 1:5, :],
                )

                # ---- y-pass (window 7 over YP=72 padded -> D=64)
                m2y = ytp.tile([128, G * ZS * YP], dt)
                m2yv = m2y[:].rearrange("p (g z y) -> p g z y", g=G, z=ZS)
                nc.vector.tensor_max(
                    m2yv[:, :, :, 0:YP - 2], z8v[:, :, :, 0:YP - 2],
                    z8v[:, :, :, 1:YP - 1],
                )
                m4y = ytp.tile([128, G * ZS * YP], dt)
                m4yv = m4y[:].rearrange("p (g z y) -> p g z y", g=G, z=ZS)
                nc.vector.tensor_max(
                    m4yv[:, :, :, 0:YP - 4], m2yv[:, :, :, 0:YP - 4],
                    m2yv[:, :, :, 2:YP - 2],
                )
                oy = oyp.tile([128, PB * VOX2], dt)
                oyv = oy[:].rearrange("p (g z y) -> p g z y", g=G, z=ZS)
                nc.vector.tensor_max(
                    oyv, m4yv[:, :, :, 1:1 + D], m4yv[:, :, :, 4:4 + D]
                )
                for b in range(PB):
                    oy_t[i + b] = oy[:, b * VOX2:(b + 1) * VOX2]
                oyP_t[i // 2] = oy

            # ---- x-pass (pair-based window-7: 2.5 ops/plane):
            #   A[p] = max(oy[2p], oy[2p+1]);  B[p] = max(A[p-1], A[p])
            #   C[p] = max(B[p], A[p+1])   (= A[p-1] | A[p] | A[p+1])
            #   win(2q)   = max(oy[2q-3], C[q])
            #   win(2q+1) = max(C[q], oy[2q+4])
            # missing terms are -inf and handled by aliasing
            if i % 2 == 0:
                pi2 = i // 2
                if pi2 < NX // 2:
                    oyP = oyP_t.pop(pi2)
                    A = axp.tile([128, VOX2], dt)
                    nc.vector.tensor_max(
                        A[:], oyP[:, 0:VOX2], oyP[:, VOX2:2 * VOX2]
                    )
                    A_t[pi2] = A[:]
                B_t[pi2] = max2(bxp, A_t.get(pi2 - 1), A_t.get(pi2))
                if pi2 >= 1:
                    C_t[pi2 - 1] = max2(cxp, B_t.get(pi2 - 1), A_t.get(pi2))
                for k, ca, oj in ((i - 2, C_t.get(pi2 - 1), i - 5),
                                  (i - 3, C_t.get(pi2 - 2), i)):
                    if 0 <= k < NX:
                        emit_mlp(k, max2(pxp, ca, oy_t.get(oj)))

            # ---- dense pair multiply (delayed so the PE/ACT round trip
            # never blocks the in-order DVE queue): prod = sg * plane_center
            if i % 2 == 0 and 0 <= i - 6 < NX:
                q = (i - 6) // 2
                sgP = sg_t.pop(q)
                Pc = Pm_t.pop(q).rearrange("p (g z y) -> p g z y", g=G, z=ZH)[
                    :, :, 3:3 + ZS, :
                ]
                prod = prp.tile([128, 2 * VOX2], dt)
                prodv = prod[:].rearrange("p (g z y) -> p g z y", g=G, z=ZS)
                nc.vector.tensor_mul(prodv, sgP[:].rearrange(
                    "p (g z y) -> p g z y", g=G, z=ZS), Pc)
                nc.gpsimd.dma_start(
                    outp[2 * q * 128:(2 * q + 2) * 128, :].rearrange(
                        "(e p) v -> p e v", p=128),
                    prod[:].rearrange("p (e v) -> p e v", e=2),
                )

    nc.compile()
    return nc


def host_prep(cfg: Cfg, feats, coords, W1, W2):
    """Build per-core dense slabs in device layout. Pure data marshalling."""
    D, ZS, ZH, NX = cfg.D, cfg.ZS, cfg.ZH, cfg.NX
    dt = _np_dt(cfg.dt)

    ix = coords[:, 0].astype(np.int64)
    iy = coords[:, 1].astype(np.int64)
    iz = coords[:, 2].astype(np.int64)

    # dense grid, padded z by 3 each side, layout [x, p, h, zpad, y]
    gridT = np.full((D, 128, 2, D + 6, D), SENT, dtype=dt)
    fsplit = feats.astype(dt).reshape(-1, 2, 128)  # [N, h, p]
    gridT[ix, :, :, iz + 3, iy] = fsplit.transpose(0, 2, 1)

    w1h = np.ascontiguousarray(W1.astype(dt))
    w2h = np.ascontiguousarray(W2.astype(dt))

    in_maps = []
    for k in range(cfg.ncores):
        slab = np.ascontiguousarray(gridT[:, :, :, 8 * k:8 * k + ZH, :])
        in_maps.append({
            "planes": slab.reshape(NX * 128, cfg.PLF),
            "w1": w1h,
            "w2": w2h,
        })
    return in_maps


def host_post(cfg: Cfg, results, coords):
    """Gather occupied voxels from the dense product planes."""
    D, ZS, NX = cfg.D, cfg.ZS, cfg.NX
    ix = coords[:, 0].astype(np.int64)
    iy = coords[:, 1].astype(np.int64)
    iz = coords[:, 2].astype(np.int64)
    out = np.empty((cfg.NPTS, C), np.float32)
    for k in range(cfg.ncores):
        sel = np.where((iz >= k * ZS) & (iz < (k + 1) * ZS))[0]
        pk = np.asarray(results[k]["outp"]).reshape(NX, 128, 2, ZS, D)
        # value for point n at channel c = h*128+p: pk[ix, p, h, iz%8, iy]
        v = pk[ix[sel], :, :, iz[sel] - k * ZS, iy[sel]]  # [n, 128, 2]
        out[sel] = v.transpose(0, 2, 1).reshape(len(sel), C).astype(np.float32)
    return out


_CACHE = {}


def _get_nc(cfg: Cfg):
    if cfg not in _CACHE:
        _CACHE[cfg] = build_nc(cfg)
    return _CACHE[cfg]


def kernel(feats, coords, W1, W2):
    from concourse.bass_utils import run_bass_kernel_spmd

    cfg = FULL
    nc = _get_nc(cfg)
    in_maps = host_prep(
        cfg,
        np.asarray(feats, np.float32),
        np.asarray(coords),
        np.asarray(W1, np.float32),
        np.asarray(W2, np.float32),
    )
    res = run_bass_kernel_spmd(nc, in_maps, core_ids=list(range(cfg.ncores)))
    return host_post(cfg, res.results, np.asarray(coords))


# revision 54
# speedup vs baseline: 1.0004x; 1.0004x over previous
"""Trainium2 Bass kernel: sparse 7x7x7 stride-1 max-pool over a 64^3 voxel grid
(MinkowskiEngine semantics) + per-point MLP (1x1 conv -> ReLU -> 1x1 conv ->
sigmoid) * feats.

Strategy (8 NeuronCores, SPMD, no collectives):
  - Shard the dense grid along z: core k owns z in [8k, 8k+8), and processes a
    14-plane z-slab (3-voxel halo each side; halo replicated on host -> no
    cross-core exchange).
  - The HOST builds the dense slab in the exact on-chip layout the kernel
    wants: per x-plane [128 part = ch%128, 2 ch-halves, 14 z, 64 y] with
    empty voxels = -1e30. Pure data marshalling (scatter + transpose + pad);
    all max/matmul arithmetic stays on device.
  - Device streams over the 64 x-planes: contiguous plane load; window-7 max
    on DVE along z via a 5-op pairs-and-singles scheme (A=z-pair maxes,
    A3=triples of pairs, then even/odd outputs from A3 + one raw plane,
    saving 19% FD vs the plain m2/m4/m7 chain), writing into a z8 tile whose
    y-borders are filled by tiny SBUF->SBUF DMAs on the sync queue; window-7
    along y on the padded z8 (m2/m4/m7 chain); window-7 along x via the
    streaming pair scheme A[p]=max(oy2p,oy2p+1), B=A|A', C=B|A'' and
    win(2q)=max(oy[2q-3],C_q), win(2q+1)=max(C_q,oy[2q+4]) -- 2.4 ops/plane
    instead of 3; fused MLP on PE with the second matmul transposed (W2^T as
    weights) so sigmoid lands in [ch, vox] layout, written into pair tiles;
    one dense pair multiply sg * plane-centers on DVE + one pair store.
  - Host gathers the occupied voxels from the dense product planes (reverse
    marshalling) and casts to fp32.

Measured on the 8-core axon TRN2 fleet: HW exec ~397 us (baseline scatter/
transpose design: 870-895 us; first host-marshalled rewrite: 459 us), rel
err 6.4e-3 vs the fp32 reference. The kernel is DVE-bound: ~405 us of
tensor_max work at ~100% vector-engine occupancy (Scalar ~33%, PE ~37%, DMA
queues <50%); pair-batched z/y passes, 2-pair DMA prefetch and a delayed
pair multiply keep everything else off the critical path. Ops compute only
liveness-required rows/cols (zB 5 of 6 rows, m2y/m4y trimmed) in full-size
tiles so SBUF addresses stay fixed. Notes: (1) DVE op throughput is
sensitive to SBUF operand address phase -- pool sizes/order below are an
empirical local optimum; seemingly neutral layout changes (y-pad 70 vs 72,
+32-elem tile stagger, 3-deep prefetch) each cost 80-90 us. (2) GPSIMD
elementwise offload is counterproductive: ~7 us/op and SBUF port contention
slows concurrent DVE ops by ~30%. (3) 5D access patterns with size-1 dims
crash the device; use a double-width innermost dim instead. (4) An odd
(2-byte) operand offset costs ~27% on strided multi-run ops but a full 2x
(hard 1x mode) on flat single-run ops -- keep odd-offset operands in
run-structured APs.
"""

from contextlib import ExitStack
from dataclasses import dataclass

import numpy as np

C = 256
R = 128
SENT = -1.0e30


@dataclass(frozen=True)
class Cfg:
    D: int = 64           # grid extent per axis
    ZS: int = 8           # owned z-planes per core
    NPTS: int = 100000    # total points
    ncores: int = 8
    dt: str = "bfloat16"  # dtype of planes / pooling / matmul inputs
    PB: int = 2           # planes per build batch (z/y pass batching)

    @property
    def ZH(self):
        return self.ZS + 6

    @property
    def YP(self):
        return self.D + 8  # y padded to 72 (4 sentinel cols each side)

    @property
    def NX(self):
        return self.D

    @property
    def PLF(self):
        return 2 * self.ZH * self.D  # free elems per partition per plane

    @property
    def VOX2(self):
        return 2 * self.ZS * self.D  # compact pooled free elems (2h * 8z * 64y)


FULL = Cfg()


def _np_dt(name):
    import ml_dtypes

    return {"bfloat16": ml_dtypes.bfloat16, "float16": np.float16}[name]


def build_nc(cfg: Cfg):
    """Build the (SPMD, per-core-identical) Bass program."""
    import concourse.bacc as bacc
    import concourse.tile as tile
    from concourse import mybir

    AF = mybir.ActivationFunctionType
    f32 = mybir.dt.float32
    dt = getattr(mybir.dt, cfg.dt)

    D, ZS, ZH, YP, NX, PB = cfg.D, cfg.ZS, cfg.ZH, cfg.YP, cfg.NX, cfg.PB
    PLF = cfg.PLF
    VOX2 = cfg.VOX2
    VOXH = ZS * D  # voxels per x-plane owned (512)
    G = 2 * PB     # merged (plane, half) dim in batched views

    nc = bacc.Bacc("TRN2", target_bir_lowering=False, debug=False,
                   enable_asserts=False, num_devices=cfg.ncores)

    planes = nc.dram_tensor("planes", [NX * 128, PLF], dt, kind="ExternalInput").ap()
    w1 = nc.dram_tensor("w1", [C, R], dt, kind="ExternalInput").ap()
    w2 = nc.dram_tensor("w2", [R, C], dt, kind="ExternalInput").ap()
    outp = nc.dram_tensor("outp", [NX * 128, VOX2], dt, kind="ExternalOutput").ap()

    with tile.TileContext(nc) as tc, ExitStack() as ctx:
        const = ctx.enter_context(tc.tile_pool(name="const", bufs=1))
        pp = ctx.enter_context(tc.tile_pool(name="pp", bufs=6))
        ztp = ctx.enter_context(tc.tile_pool(name="ztp", bufs=2))
        ytp = ctx.enter_context(tc.tile_pool(name="ytp", bufs=2))
        oyp = ctx.enter_context(tc.tile_pool(name="oyp", bufs=4))
        axp = ctx.enter_context(tc.tile_pool(name="axp", bufs=3))
        bxp = ctx.enter_context(tc.tile_pool(name="bxp", bufs=3))
        cxp = ctx.enter_context(tc.tile_pool(name="cxp", bufs=3))
        pxp = ctx.enter_context(tc.tile_pool(name="pxp", bufs=4))
        hpp = ctx.enter_context(tc.tile_pool(name="hpp", bufs=2, space="PSUM"))
        y2p = ctx.enter_context(tc.tile_pool(name="y2p", bufs=2, space="PSUM"))
        hsp = ctx.enter_context(tc.tile_pool(name="hsp", bufs=3))
        sgp = ctx.enter_context(tc.tile_pool(name="sgp", bufs=3))
        prp = ctx.enter_context(tc.tile_pool(name="prp", bufs=2))

        # ---- constants (allocate first so SBUF layout is stable; the
        # weight DMAs are issued after the first plane loads below)
        w1sb = const.tile([128, 2 * R], dt)
        w2sb = const.tile([128, C], dt)
        neg = const.tile([128, VOX2], dt)
        w1v = w1sb[:].rearrange("p (h r) -> p h r", h=2)

        # rings hold APs (slices of pool tiles); missing entries mean -inf,
        # handled by aliasing (max(x, -inf) = x)
        oy_t, oyP_t = {}, {}
        A_t, B_t, C_t = {}, {}, {}

        Pm_t = {}  # pair plane views for the final multiply
        Pp_t = {}  # prefetched pair tiles
        sg_t = {}  # pair sigmoid tiles awaiting the delayed multiply

        def max2(pool, a, b):
            """tensor_max with -inf aliasing: either operand may be None."""
            if a is None:
                return b
            if b is None:
                return a
            t = pool.tile([128, VOX2], dt)
            nc.vector.tensor_max(t[:], a, b)
            return t[:]

        def emit_mlp(k, pxa):
            # ---- MLP on plane k: h = relu(W1^T @ px) on PE+ACT
            pxv = pxa.rearrange("p (h v) -> p h v", h=2)
            hp = hpp.tile([128, VOXH], f32, space="PSUM")
            for h in (0, 1):
                nc.tensor.matmul(
                    hp[:], w1v[:, h, :], pxv[:, h, :],
                    start=(h == 0), stop=(h == 1),
                )
            hs = hsp.tile([128, VOXH], dt)
            nc.scalar.activation(hs[:], hp[:], AF.Relu)
            # ---- y2^T = W2^T @ h: output lands [ch-part, vox]
            y2 = y2p.tile([128, 2 * VOXH], f32, space="PSUM")
            for h in (0, 1):
                nc.tensor.matmul(
                    y2[:, h * VOXH:(h + 1) * VOXH],
                    w2sb[:, h * 128:(h + 1) * 128], hs[:],
                    start=True, stop=True,
                )
            if k % 2 == 0:
                sg_t[k // 2] = sgp.tile([128, 2 * VOX2], dt, name="sgP")
            sgP = sg_t[k // 2]
            nc.scalar.activation(
                sgP[:, (k % 2) * VOX2:(k % 2 + 1) * VOX2], y2[:], AF.Sigmoid
            )

        def load_pair(pi):
            P = pp.tile([128, PB * PLF], dt)
            for b in range(PB):
                x = pi * PB + b
                nc.sync.dma_start(
                    P[:, b * PLF:(b + 1) * PLF],
                    planes[x * 128:(x + 1) * 128, :],
                )
            Pm_t[pi] = P[:]
            Pp_t[pi] = P

        for pi0 in range(2):
            load_pair(pi0)
        # weight loads + sentinel memset issue behind the first plane loads
        nc.sync.dma_start(
            w1sb[:].rearrange("p (h r) -> p h r", h=2),
            w1.rearrange("(h p) r -> p h r", p=128),
        )
        nc.sync.dma_start(w2sb[:], w2)
        nc.gpsimd.memset(neg[:], SENT)

        for i in range(NX + 5):
            if i < NX and i % PB == 0:
                pi = i // PB
                if pi + 2 < NX // PB:
                    load_pair(pi + 2)
                P = Pp_t.pop(pi)
                # batched view: (pl h) merges into one uniform dim of 2*PB
                Pv = P[:].rearrange("p (g z y) -> p g z y", g=G, z=ZH)

                # ---- z-pass (window 7 over ZH=14 -> ZS=8) via z-pairs:
                #   A[m] = max(P[2m], P[2m+1]);  B[m] = max(A[m], A[m+1])
                #   A3[m] = max(B[m], A[m+2])   (= P[2m..2m+5])
                #   z8[2m]   = max(A3[m], P[2m+6])
                #   z8[2m+1] = max(P[2m+1], A3[m+1])
                P4 = P[:].rearrange("p (g m y2) -> p g m y2", g=G, m=ZH // 2)
                zA = ztp.tile([128, G * 7 * D], dt)
                zAv = zA[:].rearrange("p (g m y) -> p g m y", g=G, m=7)
                nc.vector.tensor_max(
                    zAv, P4[:, :, :, 0:D], P4[:, :, :, D:2 * D]
                )
                zB = ztp.tile([128, G * 6 * D], dt)
                zBv = zB[:].rearrange("p (g m y) -> p g m y", g=G, m=6)
                nc.vector.tensor_max(
                    zBv[:, :, 0:5, :], zAv[:, :, 0:5, :], zAv[:, :, 1:6, :]
                )
                zA3 = ztp.tile([128, G * 5 * D], dt)
                zA3v = zA3[:].rearrange("p (g m y) -> p g m y", g=G, m=5)
                nc.vector.tensor_max(
                    zA3v, zBv[:, :, 0:5, :], zAv[:, :, 2:7, :]
                )
                z8 = ztp.tile([128, G * ZS * YP], dt)
                z8v = z8[:].rearrange("p (g z y) -> p g z y", g=G, z=ZS)
                z8r = z8[:].rearrange("p (g m y2) -> p g m y2", g=G, m=ZS // 2)
                negb = neg[:, 0:G * ZS * 4].rearrange(
                    "p (g z y) -> p g z y", g=G, z=ZS)
                nc.sync.dma_start(z8v[:, :, :, 0:4], negb)
                nc.sync.dma_start(z8v[:, :, :, 4 + D:YP], negb)
                nc.vector.tensor_max(
                    z8r[:, :, :, 4:4 + D],
                    zA3v[:, :, 0:4, :], P4[:, :, 3:7, 0:D],
                )
                nc.vector.tensor_max(
                    z8r[:, :, :, YP + 4:YP + 4 + D],
                    P4[:, :, 0:4, D:2 * D], zA3v[:, :, 1:5, :],
                )

                # ---- y-pass (window 7 over YP=72 padded -> D=64)
                m2y = ytp.tile([128, G * ZS * YP], dt)
                m2yv = m2y[:].rearrange("p (g z y) -> p g z y", g=G, z=ZS)
                nc.vector.tensor_max(
                    m2yv[:, :, :, 0:YP - 2], z8v[:, :, :, 0:YP - 2],
                    z8v[:, :, :, 1:YP - 1],
                )
                m4y = ytp.tile([128, G * ZS * YP], dt)
                m4yv = m4y[:].rearrange("p (g z y) -> p g z y", g=G, z=ZS)
                nc.vector.tensor_max(
                    m4yv[:, :, :, 0:YP - 4], m2yv[:, :, :, 0:YP - 4],
                    m2yv[:, :, :, 2:YP - 2],
                )
                oy = oyp.tile([128, PB * VOX2], dt)
                oyv = oy[:].rearrange("p (g z y) -> p g z y", g=G, z=ZS)
                nc.vector.tensor_max(
                    oyv, m4yv[:, :, :, 1:1 + D], m4yv[:, :, :, 4:4 + D]
                )
                for b in range(PB):
                    oy_t[i + b] = oy[:, b * VOX2:(b + 1) * VOX2]
                oyP_t[i // 2] = oy

            # ---- x-pass (pair-based window-7: 2.5 ops/plane):
            #   A[p] = max(oy[2p], oy[2p+1]);  B[p] = max(A[p-1], A[p])
            #   C[p] = max(B[p], A[p+1])   (= A[p-1] | A[p] | A[p+1])
            #   win(2q)   = max(oy[2q-3], C[q])
            #   win(2q+1) = max(C[q], oy[2q+4])
            # missing terms are -inf and handled by aliasing
            if i % 2 == 0:
                pi2 = i // 2
                if pi2 < NX // 2:
                    oyP = oyP_t.pop(pi2)
                    A = axp.tile([128, VOX2], dt)
                    nc.vector.tensor_max(
                        A[:], oyP[:, 0:VOX2], oyP[:, VOX2:2 * VOX2]
                    )
                    A_t[pi2] = A[:]
                B_t[pi2] = max2(bxp, A_t.get(pi2 - 1), A_t.get(pi2))
                if pi2 >= 1:
                    C_t[pi2 - 1] = max2(cxp, B_t.get(pi2 - 1), A_t.get(pi2))
                for k, ca, oj in ((i - 2, C_t.get(pi2 - 1), i - 5),
                                  (i - 3, C_t.get(pi2 - 2), i)):
                    if 0 <= k < NX:
                        emit_mlp(k, max2(pxp, ca, oy_t.get(oj)))

            # ---- dense pair multiply (delayed so the PE/ACT round trip
            # never blocks the in-order DVE queue): prod = sg * plane_center
            if i % 2 == 0 and 0 <= i - 6 < NX - 4:
                q = (i - 6) // 2
                sgP = sg_t.pop(q)
                Pc = Pm_t.pop(q).rearrange("p (g z y) -> p g z y", g=G, z=ZH)[
                    :, :, 3:3 + ZS, :
                ]
                prod = prp.tile([128, 2 * VOX2], dt)
                prodv = prod[:].rearrange("p (g z y) -> p g z y", g=G, z=ZS)
                nc.vector.tensor_mul(prodv, sgP[:].rearrange(
                    "p (g z y) -> p g z y", g=G, z=ZS), Pc)
                nc.gpsimd.dma_start(
                    outp[2 * q * 128:(2 * q + 2) * 128, :].rearrange(
                        "(e p) v -> p e v", p=128),
                    prod[:].rearrange("p (e v) -> p e v", e=2),
                )
            # last 4 planes: single multiplies fire per-sigmoid to fill the
            # pipeline tail instead of waiting for a full pair
            k2 = i - 5
            if NX - 4 <= k2 < NX:
                q1 = k2 // 2
                b = k2 % 2
                sgP = sg_t[q1]
                Pc1 = Pm_t[q1].rearrange("p (g z y) -> p g z y", g=G, z=ZH)[
                    :, 2 * b:2 * b + 2, 3:3 + ZS, :
                ]
                prods = prp.tile([128, VOX2], dt, name="prods")
                prodsv = prods[:].rearrange("p (h z y) -> p h z y", h=2, z=ZS)
                nc.vector.tensor_mul(
                    prodsv,
                    sgP[:, b * VOX2:(b + 1) * VOX2].rearrange(
                        "p (h z y) -> p h z y", h=2, z=ZS),
                    Pc1,
                )
                nc.gpsimd.dma_start(
                    outp[k2 * 128:(k2 + 1) * 128, :], prods[:]
                )

    nc.compile()
    return nc


def host_prep(cfg: Cfg, feats, coords, W1, W2):
    """Build per-core dense slabs in device layout. Pure data marshalling."""
    D, ZS, ZH, NX = cfg.D, cfg.ZS, cfg.ZH, cfg.NX
    dt = _np_dt(cfg.dt)

    ix = coords[:, 0].astype(np.int64)
    iy = coords[:, 1].astype(np.int64)
    iz = coords[:, 2].astype(np.int64)

    # dense grid, padded z by 3 each side, layout [x, p, h, zpad, y]
    gridT = np.full((D, 128, 2, D + 6, D), SENT, dtype=dt)
    fsplit = feats.astype(dt).reshape(-1, 2, 128)  # [N, h, p]
    gridT[ix, :, :, iz + 3, iy] = fsplit.transpose(0, 2, 1)

    w1h = np.ascontiguousarray(W1.astype(dt))
    w2h = np.ascontiguousarray(W2.astype(dt))

    in_maps = []
    for k in range(cfg.ncores):
        slab = np.ascontiguousarray(gridT[:, :, :, 8 * k:8 * k + ZH, :])
        in_maps.append({
            "planes": slab.reshape(NX * 128, cfg.PLF),
            "w1": w1h,
            "w2": w2h,
        })
    return in_maps


def host_post(cfg: Cfg, results, coords):
    """Gather occupied voxels from the dense product planes."""
    D, ZS, NX = cfg.D, cfg.ZS, cfg.NX
    ix = coords[:, 0].astype(np.int64)
    iy = coords[:, 1].astype(np.int64)
    iz = coords[:, 2].astype(np.int64)
    out = np.empty((cfg.NPTS, C), np.float32)
    for k in range(cfg.ncores):
        sel = np.where((iz >= k * ZS) & (iz < (k + 1) * ZS))[0]
        pk = np.asarray(results[k]["outp"]).reshape(NX, 128, 2, ZS, D)
        # value for point n at channel c = h*128+p: pk[ix, p, h, iz%8, iy]
        v = pk[ix[sel], :, :, iz[sel] - k * ZS, iy[sel]]  # [n, 128, 2]
        out[sel] = v.transpose(0, 2, 1).reshape(len(sel), C).astype(np.float32)
    return out


_CACHE = {}


def _get_nc(cfg: Cfg):
    if cfg not in _CACHE:
        _CACHE[cfg] = build_nc(cfg)
    return _CACHE[cfg]


def kernel(feats, coords, W1, W2):
    from concourse.bass_utils import run_bass_kernel_spmd

    cfg = FULL
    nc = _get_nc(cfg)
    in_maps = host_prep(
        cfg,
        np.asarray(feats, np.float32),
        np.asarray(coords),
        np.asarray(W1, np.float32),
        np.asarray(W2, np.float32),
    )
    res = run_bass_kernel_spmd(nc, in_maps, core_ids=list(range(cfg.ncores)))
    return host_post(cfg, res.results, np.asarray(coords))
